# revision 7
# baseline (speedup 1.0000x reference)
"""Causal GQA self-attention (B=4, S=2048, D=2048, 16 Q heads / 4 KV heads,
RoPE) on 8 Trainium2 NeuronCores.

Sharding: tensor-parallel over heads, TP=8. Core t owns Q heads {2t, 2t+1}
and KV head t//2, and wo's in-feature columns [256t, 256(t+1)). Every core
processes all 4 batches sequentially. Host sums the 8 wo partial outputs.

Device layouts (feature-major so the matmul contraction dim sits on SBUF
partitions; zero on-device transposes except V):
  xT   [B, D, S]    x transposed per batch (host-side)
  wqT  [D, 256]     wq shard transposed   -> Q^T = wqT.T @ xT
  wkT/wvT [D, 128]; woT [256, D]
  Q^T/K^T [head_dim, s] with RoPE applied in-layout; rotate-half done with
  two cross-partition copies + stacked tables C=[cos;cos], S2=[sin;-sin]
  scores computed directly transposed: S^T[k, q] = Krot^T.T @ Qrot^T
  causal diag masking: additive -1e30 bias into PSUM before exp
  softmax denominator via all-ones [128,128] lhsT matmul (PSUM-accumulated
  broadcast column sums); no max subtraction (logits are O(1))
  AV: y^T[dd, q] = V.T @ expS^T with V in natural [s, dd] layout (V is
  produced as V^T then PE-transposed once)
  O-proj emits outT [B, D, S]; host sums partials over cores + transposes.

Matmul operands are float32r end-to-end (full PE rate at free-dim >= 256):
DRAM inputs + SBUF tiles are declared float32r (bitwise fp32), computed
operands (Qrot/Krot/V/expS/Y) get their f32r rounding from the DVE/ACT op
that produces them.
"""

import sys
from contextlib import ExitStack

import numpy as np

for _p in ("/opt/trn_rl_repo", "/root/.axon_site/_ro/trn_rl_repo"):
    if _p not in sys.path:
        sys.path.append(_p)

import concourse.bass as bass  # noqa: E402
import concourse.tile as tile  # noqa: E402
from concourse import bacc, mybir  # noqa: E402
from concourse.bass_utils import run_bass_kernel_spmd  # noqa: E402

F32 = mybir.dt.float32
F32R = mybir.dt.float32r
EXP = mybir.ActivationFunctionType.Exp

B, S, D = 4, 2048, 2048
HD = 128          # head dim
QH = 2            # q heads per core
EQ = QH * HD      # 256: q-proj out features per core
SC = 512          # seq chunk
DT = D // 128     # 16 contraction tiles
SCALE = 1.0 / float(np.sqrt(HD))
NEG = -1.0e30
N_CORES = 8


def _rope(nc, pool, out, ps, Ct, S2t, w):
    """out = ps*C + rotate_half(ps)*S2 in [head_dim, w] layout.
    rotate_half swaps the partition halves; C=[cos;cos], S2=[sin;-sin].
    out is an f32r tile (the add performs the f32r rounding)."""
    rot = pool.tile([128, w], F32, tag="rope_rot", bufs=3)
    nc.vector.tensor_copy(rot[0:64, :], ps[64:128, :])
    nc.vector.tensor_copy(rot[64:128, :], ps[0:64, :])
    ta = pool.tile([128, w], F32, tag="rope_a", bufs=3)
    nc.vector.tensor_mul(ta, ps, Ct)
    tb = pool.tile([128, w], F32, tag="rope_b", bufs=3)
    nc.vector.tensor_mul(tb, rot, S2t)
    nc.vector.tensor_add(out, ta, tb)


def build(b_count=B, seq=S):
    """Build + compile the per-core program. Identical across cores (SPMD);
    all TP-rank differences live in the data."""
    nch = seq // SC
    nc = bacc.Bacc("TRN2", target_bir_lowering=False, debug=False,
                   num_devices=N_CORES)

    xT = nc.dram_tensor("xT", [b_count, D, seq], F32R, kind="ExternalInput").ap()
    wqT = nc.dram_tensor("wqT", [D, EQ], F32R, kind="ExternalInput").ap()
    wkT = nc.dram_tensor("wkT", [D, HD], F32R, kind="ExternalInput").ap()
    wvT = nc.dram_tensor("wvT", [D, HD], F32R, kind="ExternalInput").ap()
    woT = nc.dram_tensor("woT", [EQ, D], F32R, kind="ExternalInput").ap()
    Cd = nc.dram_tensor("C", [128, seq], F32, kind="ExternalInput").ap()
    S2d = nc.dram_tensor("S2", [128, seq], F32, kind="ExternalInput").ap()
    masks = nc.dram_tensor("masks", [128, 4, SC], F32, kind="ExternalInput").ap()
    ident = nc.dram_tensor("ident", [128, 128], F32, kind="ExternalInput").ap()
    ones = nc.dram_tensor("ones", [128, 128], F32R, kind="ExternalInput").ap()
    outT = nc.dram_tensor("outT", [b_count, D, seq], F32,
                          kind="ExternalOutput").ap()

    with tile.TileContext(nc) as tc, ExitStack() as ctx:
        pool = ctx.enter_context(tc.tile_pool(name="sb", bufs=2))
        psum = ctx.enter_context(tc.tile_pool(name="ps", bufs=2, space="PSUM"))

        # resident weights / tables (wq split in two DMAs so the first
        # projection matmuls can start sooner)
        wq_sb = pool.tile([128, DT, EQ], F32R, tag="wq", bufs=1)
        wqT_r = wqT.rearrange("(dt p) e -> p dt e", p=128)
        nc.sync.dma_start(wq_sb[:, 0:DT // 2, :], wqT_r[:, 0:DT // 2, :])
        nc.sync.dma_start(wq_sb[:, DT // 2:, :], wqT_r[:, DT // 2:, :])
        wk_sb = pool.tile([128, DT, HD], F32R, tag="wk", bufs=1)
        nc.sync.dma_start(wk_sb, wkT.rearrange("(dt p) e -> p dt e", p=128))
        wv_sb = pool.tile([128, DT, HD], F32R, tag="wv", bufs=1)
        nc.sync.dma_start(wv_sb, wvT.rearrange("(dt p) e -> p dt e", p=128))
        wo_sb = pool.tile([128, QH, D], F32R, tag="wo", bufs=1)
        nc.sync.dma_start(wo_sb, woT.rearrange("(h p) e -> p h e", p=128))
        C_sb = pool.tile([128, seq], F32, tag="C", bufs=1)
        nc.sync.dma_start(C_sb, Cd)
        S2_sb = pool.tile([128, seq], F32, tag="S2", bufs=1)
        nc.sync.dma_start(S2_sb, S2d)
        mask_sb = pool.tile([128, 4, SC], F32, tag="mask", bufs=1)
        nc.sync.dma_start(mask_sb, masks)
        id_sb = pool.tile([128, 128], F32, tag="id", bufs=1)
        nc.sync.dma_start(id_sb, ident)
        ones_sb = pool.tile([128, 128], F32R, tag="ones", bufs=1)
        nc.sync.dma_start(ones_sb, ones)

        for b in range(b_count):
            krot = pool.tile([128, seq], F32R, tag="krot", bufs=2)
            v_sb = pool.tile([128, seq // 128, HD], F32R, tag="v", bufs=2)

            for c in range(nch):
                s0 = c * SC
                C_c = C_sb[:, s0:s0 + SC]
                S2_c = S2_sb[:, s0:s0 + SC]

                xts = []
                for dt in range(DT):
                    t = pool.tile([128, SC], F32R, tag="xt", bufs=18)
                    nc.sync.dma_start(t, xT[b, 128 * dt:128 * (dt + 1),
                                            s0:s0 + SC])
                    xts.append(t)

                # ---- Q projection + RoPE (2 heads) ----
                qts = []
                for h in range(QH):
                    ps = psum.tile([128, SC], F32, tag="proj", bufs=2)
                    for dt in range(DT):
                        nc.tensor.matmul(ps, wq_sb[:, dt, HD * h:HD * (h + 1)],
                                         xts[dt],
                                         start=(dt == 0), stop=(dt == DT - 1))
                    qt = pool.tile([128, SC], F32R, tag="qrot", bufs=6)
                    _rope(nc, pool, qt, ps, C_c, S2_c, SC)
                    qts.append(qt)

                # ---- K projection + RoPE into resident K cache ----
                psk = psum.tile([128, SC], F32, tag="proj", bufs=2)
                for dt in range(DT):
                    nc.tensor.matmul(psk, wk_sb[:, dt, :], xts[dt],
                                     start=(dt == 0), stop=(dt == DT - 1))
                _rope(nc, pool, krot[:, s0:s0 + SC], psk, C_c, S2_c, SC)

                # ---- V^T projection, then PE-transpose into natural V ----
                psv = psum.tile([128, SC], F32, tag="proj", bufs=2)
                for dt in range(DT):
                    nc.tensor.matmul(psv, wv_sb[:, dt, :], xts[dt],
                                     start=(dt == 0), stop=(dt == DT - 1))
                vtmp = pool.tile([128, SC], F32, tag="vtmp", bufs=2)
                nc.vector.tensor_copy(vtmp, psv)
                for st in range(SC // 128):
                    pst = psum.tile([128, 128], F32, tag="proj", bufs=2)
                    nc.tensor.transpose(pst, vtmp[:, 128 * st:128 * (st + 1)],
                                        id_sb)
                    nc.vector.tensor_copy(v_sb[:, 4 * c + st, :], pst)

                # ---- causal attention for this q-chunk ----
                # Heads interleaved per k-tile so one head's exp (ACT)
                # hides under the other head's matmuls (PE).
                nkt = 4 * (c + 1)
                sum_ps = [psum.tile([128, SC], F32, tag="sum", bufs=2,
                                    name=f"sum_{b}_{c}_{h}")
                          for h in range(QH)]
                y_ps = [psum.tile([128, SC], F32, tag="y", bufs=2,
                                  name=f"y_{b}_{c}_{h}")
                        for h in range(QH)]
                for kt in range(nkt):
                    sts = []
                    for h in range(QH):
                        st_ps = psum.tile([128, SC], F32, tag="st", bufs=2)
                        nc.tensor.matmul(st_ps,
                                         krot[:, 128 * kt:128 * (kt + 1)],
                                         qts[h], start=True, stop=True)
                        sts.append(st_ps)
                    ess = []
                    j = kt - 4 * c
                    for h in range(QH):
                        if j >= 0:
                            # bias is 0 beyond column 128*(j+1); add prefix only
                            w = 128 * (j + 1)
                            nc.vector.tensor_add(sts[h][:, 0:w], sts[h][:, 0:w],
                                                 mask_sb[:, j, 0:w])
                        es = pool.tile([128, SC], F32R, tag="es", bufs=4)
                        nc.scalar.activation(es, sts[h], EXP, scale=SCALE)
                        ess.append(es)
                    for h in range(QH):
                        nc.tensor.matmul(sum_ps[h], ones_sb, ess[h],
                                         start=(kt == 0), stop=(kt == nkt - 1))
                        nc.tensor.matmul(y_ps[h], v_sb[:, kt, :], ess[h],
                                         start=(kt == 0), stop=(kt == nkt - 1))
                yts = []
                for h in range(QH):
                    rec = pool.tile([128, SC], F32, tag="rec", bufs=2)
                    nc.vector.reciprocal(rec, sum_ps[h])
                    yt = pool.tile([128, SC], F32R, tag="yt", bufs=4)
                    nc.vector.tensor_mul(yt, y_ps[h], rec)
                    yts.append(yt)

                # ---- O projection (partial over this core's 256 features) ----
                for et in range(D // 128):
                    po = psum.tile([128, SC], F32, tag="proj", bufs=2)
                    for h in range(QH):
                        nc.tensor.matmul(po,
                                         wo_sb[:, h, 128 * et:128 * (et + 1)],
                                         yts[h],
                                         start=(h == 0), stop=(h == QH - 1))
                    ot = pool.tile([128, SC], F32, tag="ot", bufs=3)
                    nc.vector.tensor_copy(ot, po)
                    nc.sync.dma_start(outT[b, 128 * et:128 * (et + 1),
                                           s0:s0 + SC], ot)

    nc.compile()
    return nc


def host_tables(seq=S):
    inv = 1.0 / (10000.0 ** (np.arange(0, HD, 2, dtype=np.float64) / HD))
    t = np.arange(seq, dtype=np.float64)
    fr = np.outer(t, inv)
    cosT = np.cos(fr).T.astype(np.float32)   # [64, seq]
    sinT = np.sin(fr).T.astype(np.float32)
    C = np.concatenate([cosT, cosT], axis=0)        # [128, seq]
    S2 = np.concatenate([sinT, -sinT], axis=0)      # [128, seq]
    masks = np.zeros((128, 4, SC), np.float32)      # additive bias, 0 = keep
    for j in range(4):
        for kk in range(128):
            masks[kk, j, :min(128 * j + kk, SC)] = NEG
    return C, S2, masks


def make_in_maps(x, wq, wk, wv, wo):
    x = np.asarray(x, np.float32)
    wq = np.asarray(wq, np.float32)
    wk = np.asarray(wk, np.float32)
    wv = np.asarray(wv, np.float32)
    wo = np.asarray(wo, np.float32)
    seq = x.shape[1]
    xT = np.ascontiguousarray(x.transpose(0, 2, 1))
    C, S2, masks = host_tables(seq)
    ident = np.eye(128, dtype=np.float32)
    ones = np.ones((128, 128), np.float32)
    in_maps = []
    for t in range(N_CORES):
        g = t // 2
        in_maps.append({
            "xT": xT,
            "wqT": np.ascontiguousarray(wq[EQ * t:EQ * (t + 1)].T),
            "wkT": np.ascontiguousarray(wk[HD * g:HD * (g + 1)].T),
            "wvT": np.ascontiguousarray(wv[HD * g:HD * (g + 1)].T),
            "woT": np.ascontiguousarray(wo[:, EQ * t:EQ * (t + 1)].T),
            "C": C, "S2": S2, "masks": masks,
            "ident": ident, "ones": ones,
        })
    return in_maps


_CACHE = {}


def kernel(x, wq, wk, wv, wo, _trace=False):
    x = np.asarray(x, np.float32)
    b_count, seq = x.shape[0], x.shape[1]
    key = (b_count, seq)
    if key not in _CACHE:
        _CACHE[key] = build(b_count, seq)
    nc = _CACHE[key]
    in_maps = make_in_maps(x, wq, wk, wv, wo)
    res = run_bass_kernel_spmd(nc, in_maps, list(range(N_CORES)),
                               trace=_trace)
    acc = res.results[0]["outT"].copy()
    for t in range(1, N_CORES):
        acc += res.results[t]["outT"]
    out = np.ascontiguousarray(acc.transpose(0, 2, 1))
    if _trace:
        kernel.last_exec_time_ns = res.exec_time_ns
        kernel.last_res = res
    return out


# revision 11
# speedup vs baseline: 1.2152x; 1.2152x over previous
"""Causal GQA self-attention (B=4, S=2048, D=2048, 16 Q heads / 4 KV heads,
RoPE) on 8 Trainium2 NeuronCores.

Sharding: tensor-parallel over heads, TP=8. Core t owns Q heads {2t, 2t+1}
and KV head t//2, and wo's in-feature columns [256t, 256(t+1)). Every core
processes all 4 batches sequentially. Host sums the 8 wo partial outputs.

Device layouts (feature-major so the matmul contraction dim sits on SBUF
partitions; zero on-device transposes except V):
  xT   [B, D, S]    x transposed per batch (host-side)
  wqT  [D, 256]     wq shard transposed   -> Q^T = wqT.T @ xT
  wkT/wvT [D, 128]; woT [256, D]
  Q^T/K^T [head_dim, s] with RoPE applied in-layout; rotate-half done with
  two cross-partition copies + stacked tables C=[cos;cos], S2=[sin;-sin]
  scores computed directly transposed: S^T[k, q] = Krot^T.T @ Qrot^T
  causal diag masking: additive -1e30 bias into PSUM before exp
  softmax denominator via all-ones [128,128] lhsT matmul (PSUM-accumulated
  broadcast column sums); no max subtraction (logits are O(1))
  AV: y^T[dd, q] = V.T @ expS^T with V in natural [s, dd] layout (V is
  produced as V^T then PE-transposed once)
  O-proj emits outT [B, D, S]; host sums partials over cores + transposes.

Matmul operands are float32r end-to-end (full PE rate at free-dim >= 256):
DRAM inputs + SBUF tiles are declared float32r (bitwise fp32), computed
operands (Qrot/Krot/V/expS/Y) get their f32r rounding from the DVE/ACT op
that produces them.
"""

import sys
from contextlib import ExitStack

import numpy as np

for _p in ("/opt/trn_rl_repo", "/root/.axon_site/_ro/trn_rl_repo"):
    if _p not in sys.path:
        sys.path.append(_p)

import concourse.bass as bass  # noqa: E402
import concourse.tile as tile  # noqa: E402
from concourse import bacc, mybir  # noqa: E402
from concourse.bass_utils import run_bass_kernel_spmd  # noqa: E402

F32 = mybir.dt.float32
F32R = mybir.dt.float32r
EXP = mybir.ActivationFunctionType.Exp

B, S, D = 4, 2048, 2048
HD = 128          # head dim
QH = 2            # q heads per core
EQ = QH * HD      # 256: q-proj out features per core
SC = 512          # seq chunk
DT = D // 128     # 16 contraction tiles
SCALE = 1.0 / float(np.sqrt(HD))
NEG = -1.0e30
N_CORES = 8


def _rope(nc, pool, out, ps, Ct, S2t, w):
    """out = ps*C + rotate_half(ps)*S2 in [head_dim, w] layout.
    rotate_half swaps the partition halves; C=[cos;cos], S2=[sin;-sin].
    out is an f32r tile (the add performs the f32r rounding)."""
    rot = pool.tile([128, w], F32, tag="rope_rot", bufs=3)
    nc.vector.tensor_copy(rot[0:64, :], ps[64:128, :])
    nc.vector.tensor_copy(rot[64:128, :], ps[0:64, :])
    ta = pool.tile([128, w], F32, tag="rope_a", bufs=3)
    nc.vector.tensor_mul(ta, ps, Ct)
    tb = pool.tile([128, w], F32, tag="rope_b", bufs=3)
    nc.vector.tensor_mul(tb, rot, S2t)
    nc.vector.tensor_add(out, ta, tb)


def _emit_oproj(nc, pool, psum, wo_sb, outT, b, s0, yts):
    """Partial O projection for one (batch, chunk): outT += woT.T @ Y^T."""
    for et in range(D // 128):
        po = psum.tile([128, SC], F32, tag="proj", bufs=2,
                       name=f"po_{b}_{s0}_{et}")
        for h in range(QH):
            nc.tensor.matmul(po, wo_sb[:, h, 128 * et:128 * (et + 1)],
                             yts[h], start=(h == 0), stop=(h == QH - 1))
        ot = pool.tile([128, SC], F32, tag="ot", bufs=3,
                       name=f"ot_{b}_{s0}_{et}")
        nc.vector.tensor_copy(ot, po)
        nc.sync.dma_start(outT[b, 128 * et:128 * (et + 1), s0:s0 + SC], ot)


def build(b_count=B, seq=S):
    """Build + compile the per-core program. Identical across cores (SPMD);
    all TP-rank differences live in the data."""
    nch = seq // SC
    nc = bacc.Bacc("TRN2", target_bir_lowering=False, debug=False,
                   num_devices=N_CORES)

    xT = nc.dram_tensor("xT", [b_count, D, seq], F32R, kind="ExternalInput").ap()
    wqT = nc.dram_tensor("wqT", [D, EQ], F32R, kind="ExternalInput").ap()
    wkT = nc.dram_tensor("wkT", [D, HD], F32R, kind="ExternalInput").ap()
    wvT = nc.dram_tensor("wvT", [D, HD], F32R, kind="ExternalInput").ap()
    woT = nc.dram_tensor("woT", [EQ, D], F32R, kind="ExternalInput").ap()
    Cd = nc.dram_tensor("C", [128, seq], F32, kind="ExternalInput").ap()
    S2d = nc.dram_tensor("S2", [128, seq], F32, kind="ExternalInput").ap()
    masks = nc.dram_tensor("masks", [128, 4, SC], F32, kind="ExternalInput").ap()
    ident = nc.dram_tensor("ident", [128, 128], F32, kind="ExternalInput").ap()
    ones = nc.dram_tensor("ones", [128, 128], F32R, kind="ExternalInput").ap()
    outT = nc.dram_tensor("outT", [b_count, D, seq], F32,
                          kind="ExternalOutput").ap()

    with tile.TileContext(nc) as tc, ExitStack() as ctx:
        pool = ctx.enter_context(tc.tile_pool(name="sb", bufs=2))
        psum = ctx.enter_context(tc.tile_pool(name="ps", bufs=2, space="PSUM"))

        # resident weights / tables (wq split in two DMAs so the first
        # projection matmuls can start sooner)
        wq_sb = pool.tile([128, DT, EQ], F32R, tag="wq", bufs=1)
        wqT_r = wqT.rearrange("(dt p) e -> p dt e", p=128)
        nc.sync.dma_start(wq_sb[:, 0:DT // 2, :], wqT_r[:, 0:DT // 2, :])
        nc.sync.dma_start(wq_sb[:, DT // 2:, :], wqT_r[:, DT // 2:, :])
        wk_sb = pool.tile([128, DT, HD], F32R, tag="wk", bufs=1)
        nc.sync.dma_start(wk_sb, wkT.rearrange("(dt p) e -> p dt e", p=128))
        wv_sb = pool.tile([128, DT, HD], F32R, tag="wv", bufs=1)
        nc.sync.dma_start(wv_sb, wvT.rearrange("(dt p) e -> p dt e", p=128))
        wo_sb = pool.tile([128, QH, D], F32R, tag="wo", bufs=1)
        nc.sync.dma_start(wo_sb, woT.rearrange("(h p) e -> p h e", p=128))
        C_sb = pool.tile([128, seq], F32, tag="C", bufs=1)
        nc.sync.dma_start(C_sb, Cd)
        S2_sb = pool.tile([128, seq], F32, tag="S2", bufs=1)
        nc.sync.dma_start(S2_sb, S2d)
        mask_sb = pool.tile([128, 4, SC], F32, tag="mask", bufs=1)
        nc.sync.dma_start(mask_sb, masks)
        id_sb = pool.tile([128, 128], F32, tag="id", bufs=1)
        nc.sync.dma_start(id_sb, ident)
        ones_sb = pool.tile([128, 128], F32R, tag="ones", bufs=1)
        nc.sync.dma_start(ones_sb, ones)

        pending = []
        for b in range(b_count):
            krot = pool.tile([128, seq], F32R, tag="krot", bufs=2)
            v_sb = pool.tile([128, seq // 128, HD], F32R, tag="v", bufs=2)

            for c in range(nch):
                s0 = c * SC
                C_c = C_sb[:, s0:s0 + SC]
                S2_c = S2_sb[:, s0:s0 + SC]

                xts = []
                for dt in range(DT):
                    t = pool.tile([128, SC], F32R, tag="xt", bufs=18)
                    nc.sync.dma_start(t, xT[b, 128 * dt:128 * (dt + 1),
                                            s0:s0 + SC])
                    xts.append(t)

                # ---- Q projection + RoPE (2 heads) ----
                qts = []
                for h in range(QH):
                    ps = psum.tile([128, SC], F32, tag="proj", bufs=2)
                    for dt in range(DT):
                        nc.tensor.matmul(ps, wq_sb[:, dt, HD * h:HD * (h + 1)],
                                         xts[dt],
                                         start=(dt == 0), stop=(dt == DT - 1))
                    qt = pool.tile([128, SC], F32R, tag="qrot", bufs=6)
                    _rope(nc, pool, qt, ps, C_c, S2_c, SC)
                    qts.append(qt)

                # ---- K projection + RoPE into resident K cache ----
                psk = psum.tile([128, SC], F32, tag="proj", bufs=2)
                for dt in range(DT):
                    nc.tensor.matmul(psk, wk_sb[:, dt, :], xts[dt],
                                     start=(dt == 0), stop=(dt == DT - 1))
                _rope(nc, pool, krot[:, s0:s0 + SC], psk, C_c, S2_c, SC)

                # ---- V^T projection, then PE-transpose into natural V ----
                psv = psum.tile([128, SC], F32, tag="proj", bufs=2)
                for dt in range(DT):
                    nc.tensor.matmul(psv, wv_sb[:, dt, :], xts[dt],
                                     start=(dt == 0), stop=(dt == DT - 1))
                vtmp = pool.tile([128, SC], F32, tag="vtmp", bufs=2)
                nc.vector.tensor_copy(vtmp, psv)
                for st in range(SC // 128):
                    pst = psum.tile([128, 128], F32, tag="proj", bufs=2)
                    nc.tensor.transpose(pst, vtmp[:, 128 * st:128 * (st + 1)],
                                        id_sb)
                    nc.vector.tensor_copy(v_sb[:, 4 * c + st, :], pst)

                # ---- causal attention for this q-chunk (heads sequential:
                # head h's reciprocal tail hides under head h+1's matmuls,
                # and the last head's under the next chunk's projections
                # thanks to the deferred O-projection below) ----
                nkt = 4 * (c + 1)
                yts = []
                for h in range(QH):
                    sum_ps = psum.tile([128, SC], F32, tag="sum", bufs=2)
                    y_ps = psum.tile([128, SC], F32, tag="y", bufs=2)
                    for kt in range(nkt):
                        st_ps = psum.tile([128, SC], F32, tag="st", bufs=2)
                        nc.tensor.matmul(st_ps,
                                         krot[:, 128 * kt:128 * (kt + 1)],
                                         qts[h], start=True, stop=True)
                        j = kt - 4 * c
                        if j >= 0:
                            # bias is 0 beyond column 128*(j+1); add prefix only
                            w = 128 * (j + 1)
                            nc.vector.tensor_add(st_ps[:, 0:w], st_ps[:, 0:w],
                                                 mask_sb[:, j, 0:w])
                        es = pool.tile([128, SC], F32R, tag="es", bufs=4)
                        nc.scalar.activation(es, st_ps, EXP, scale=SCALE)
                        nc.tensor.matmul(sum_ps, ones_sb, es,
                                         start=(kt == 0), stop=(kt == nkt - 1))
                        nc.tensor.matmul(y_ps, v_sb[:, kt, :], es,
                                         start=(kt == 0), stop=(kt == nkt - 1))
                    rec = pool.tile([128, SC], F32, tag="rec", bufs=2)
                    nc.vector.reciprocal(rec, sum_ps)
                    yt = pool.tile([128, SC], F32R, tag="yt", bufs=6)
                    nc.vector.tensor_mul(yt, y_ps, rec)
                    yts.append(yt)

                # ---- deferred O projection: emit the PREVIOUS chunk's
                # O-proj here so its inputs (previous yts) are long ready
                # and this chunk's reciprocal tail hides under the next
                # chunk's projection matmuls ----
                pending.append((b, s0, yts))
                if len(pending) > 1:
                    _emit_oproj(nc, pool, psum, wo_sb, outT, *pending.pop(0))
        while pending:
            _emit_oproj(nc, pool, psum, wo_sb, outT, *pending.pop(0))

    nc.compile()
    return nc


def host_tables(seq=S):
    inv = 1.0 / (10000.0 ** (np.arange(0, HD, 2, dtype=np.float64) / HD))
    t = np.arange(seq, dtype=np.float64)
    fr = np.outer(t, inv)
    cosT = np.cos(fr).T.astype(np.float32)   # [64, seq]
    sinT = np.sin(fr).T.astype(np.float32)
    C = np.concatenate([cosT, cosT], axis=0)        # [128, seq]
    S2 = np.concatenate([sinT, -sinT], axis=0)      # [128, seq]
    masks = np.zeros((128, 4, SC), np.float32)      # additive bias, 0 = keep
    for j in range(4):
        for kk in range(128):
            masks[kk, j, :min(128 * j + kk, SC)] = NEG
    return C, S2, masks


def make_in_maps(x, wq, wk, wv, wo):
    x = np.asarray(x, np.float32)
    wq = np.asarray(wq, np.float32)
    wk = np.asarray(wk, np.float32)
    wv = np.asarray(wv, np.float32)
    wo = np.asarray(wo, np.float32)
    seq = x.shape[1]
    xT = np.ascontiguousarray(x.transpose(0, 2, 1))
    C, S2, masks = host_tables(seq)
    ident = np.eye(128, dtype=np.float32)
    ones = np.ones((128, 128), np.float32)
    in_maps = []
    for t in range(N_CORES):
        g = t // 2
        in_maps.append({
            "xT": xT,
            "wqT": np.ascontiguousarray(wq[EQ * t:EQ * (t + 1)].T),
            "wkT": np.ascontiguousarray(wk[HD * g:HD * (g + 1)].T),
            "wvT": np.ascontiguousarray(wv[HD * g:HD * (g + 1)].T),
            "woT": np.ascontiguousarray(wo[:, EQ * t:EQ * (t + 1)].T),
            "C": C, "S2": S2, "masks": masks,
            "ident": ident, "ones": ones,
        })
    return in_maps


_CACHE = {}


def kernel(x, wq, wk, wv, wo, _trace=False):
    x = np.asarray(x, np.float32)
    b_count, seq = x.shape[0], x.shape[1]
    key = (b_count, seq)
    if key not in _CACHE:
        _CACHE[key] = build(b_count, seq)
    nc = _CACHE[key]
    in_maps = make_in_maps(x, wq, wk, wv, wo)
    res = run_bass_kernel_spmd(nc, in_maps, list(range(N_CORES)),
                               trace=_trace)
    acc = res.results[0]["outT"].copy()
    for t in range(1, N_CORES):
        acc += res.results[t]["outT"]
    out = np.ascontiguousarray(acc.transpose(0, 2, 1))
    if _trace:
        kernel.last_exec_time_ns = res.exec_time_ns
        kernel.last_res = res
    return out


# revision 19
# speedup vs baseline: 1.3261x; 1.0913x over previous
"""Causal GQA self-attention (B=4, S=2048, D=2048, 16 Q heads / 4 KV heads,
RoPE) on 8 Trainium2 NeuronCores.

Sharding: tensor-parallel over heads, TP=8. Core t owns Q heads {2t, 2t+1}
and KV head t//2, and wo's in-feature columns [256t, 256(t+1)). Every core
processes all 4 batches sequentially. Host sums the 8 wo partial outputs.

Device layouts (feature-major so the matmul contraction dim sits on SBUF
partitions; zero on-device transposes except V):
  xT   [B, D, S]    x transposed per batch (host-side)
  wqT  [D, 256]     wq shard transposed   -> Q^T = wqT.T @ xT
  wkT/wvT [D, 128]; woT [256, D]
  Q^T/K^T [head_dim, s] with RoPE applied in-layout; rotate-half done with
  two cross-partition copies + stacked tables C=[cos;cos], S2=[sin;-sin]
  scores computed directly transposed: S^T[k, q] = Krot^T.T @ Qrot^T
  causal diag masking: additive -1e30 bias into PSUM before exp
  softmax denominator via all-ones [128,128] lhsT matmul (PSUM-accumulated
  broadcast column sums); no max subtraction (logits are O(1))
  AV: y^T[dd, q] = V.T @ expS^T with V in natural [s, dd] layout (V is
  produced as V^T then PE-transposed once)
  O-proj emits outT [B, D, S]; host sums partials over cores + transposes.

Matmul operands are float32r end-to-end (full PE rate at free-dim >= 256):
DRAM inputs + SBUF tiles are declared float32r (bitwise fp32), computed
operands (Qrot/Krot/V/expS/Y) get their f32r rounding from the DVE/ACT op
that produces them.
"""

import sys
from contextlib import ExitStack

import numpy as np

for _p in ("/opt/trn_rl_repo", "/root/.axon_site/_ro/trn_rl_repo"):
    if _p not in sys.path:
        sys.path.append(_p)

import concourse.bass as bass  # noqa: E402
import concourse.tile as tile  # noqa: E402
from concourse import bacc, mybir  # noqa: E402
from concourse.bass_utils import run_bass_kernel_spmd  # noqa: E402

F32 = mybir.dt.float32
F32R = mybir.dt.float32r
EXP = mybir.ActivationFunctionType.Exp

B, S, D = 4, 2048, 2048
HD = 128          # head dim
QH = 2            # q heads per core
EQ = QH * HD      # 256: q-proj out features per core
SC = 512          # seq chunk
DT = D // 128     # 16 contraction tiles
SCALE = 1.0 / float(np.sqrt(HD))
NEG = -1.0e30
N_CORES = 8


def _rope(nc, pool, out, ps, Ct, S2t, w):
    """out = ps*C + rotate_half(ps)*S2 in [head_dim, w] layout.
    rotate_half swaps the partition halves; C=[cos;cos], S2=[sin;-sin].
    out is an f32r tile (the add performs the f32r rounding).
    The rotate copies run on the Scalar engine to keep DVE free."""
    rot = pool.tile([128, w], F32, tag="rope_rot", bufs=2)
    nc.scalar.copy(rot[0:64, :], ps[64:128, :])
    nc.scalar.copy(rot[64:128, :], ps[0:64, :])
    ta = pool.tile([128, w], F32, tag="rope_a", bufs=2)
    nc.vector.tensor_mul(ta, ps, Ct)
    tb = pool.tile([128, w], F32, tag="rope_b", bufs=2)
    nc.vector.tensor_mul(tb, rot, S2t)
    nc.vector.tensor_add(out, ta, tb)


def _emit_oproj(nc, pool, psum, wo_sb, outT, b, s0, yts):
    """Partial O projection for one (batch, chunk): outT += woT.T @ Y^T."""
    for et in range(D // 128):
        po = psum.tile([128, SC], F32, tag="proj", bufs=2,
                       name=f"po_{b}_{s0}_{et}")
        for h in range(QH):
            nc.tensor.matmul(po, wo_sb[:, h, 128 * et:128 * (et + 1)],
                             yts[h], start=(h == 0), stop=(h == QH - 1))
        ot = pool.tile([128, SC], F32, tag="ot", bufs=3,
                       name=f"ot_{b}_{s0}_{et}")
        nc.scalar.copy(ot, po)
        nc.sync.dma_start(outT[b, 128 * et:128 * (et + 1), s0:s0 + SC], ot)


def build(b_count=B, seq=S):
    """Build + compile the per-core program. Identical across cores (SPMD);
    all TP-rank differences live in the data."""
    nch = seq // SC
    nc = bacc.Bacc("TRN2", target_bir_lowering=False, debug=False,
                   num_devices=N_CORES)

    xT = nc.dram_tensor("xT", [b_count, D, seq], F32R, kind="ExternalInput").ap()
    wqT = nc.dram_tensor("wqT", [D, EQ], F32R, kind="ExternalInput").ap()
    wkT = nc.dram_tensor("wkT", [D, HD], F32R, kind="ExternalInput").ap()
    wvT = nc.dram_tensor("wvT", [D, HD], F32R, kind="ExternalInput").ap()
    woT = nc.dram_tensor("woT", [EQ, D], F32R, kind="ExternalInput").ap()
    Cd = nc.dram_tensor("C", [128, seq], F32, kind="ExternalInput").ap()
    S2d = nc.dram_tensor("S2", [128, seq], F32, kind="ExternalInput").ap()
    masks = nc.dram_tensor("masks", [128, 4, SC], F32, kind="ExternalInput").ap()
    ident = nc.dram_tensor("ident", [128, 128], F32, kind="ExternalInput").ap()
    ones = nc.dram_tensor("ones", [128, 128], F32R, kind="ExternalInput").ap()
    outT = nc.dram_tensor("outT", [b_count, D, seq], F32,
                          kind="ExternalOutput").ap()

    with tile.TileContext(nc) as tc, ExitStack() as ctx:
        pool = ctx.enter_context(tc.tile_pool(name="sb", bufs=2))
        psum = ctx.enter_context(tc.tile_pool(name="ps", bufs=2, space="PSUM"))

        # resident weights / tables (wq split in two DMAs so the first
        # projection matmuls can start sooner)
        wq_sb = pool.tile([128, DT, EQ], F32R, tag="wq", bufs=1)
        wqT_r = wqT.rearrange("(dt p) e -> p dt e", p=128)
        nc.sync.dma_start(wq_sb[:, 0:DT // 2, :], wqT_r[:, 0:DT // 2, :])
        nc.sync.dma_start(wq_sb[:, DT // 2:, :], wqT_r[:, DT // 2:, :])
        wk_sb = pool.tile([128, DT, HD], F32R, tag="wk", bufs=1)
        nc.sync.dma_start(wk_sb, wkT.rearrange("(dt p) e -> p dt e", p=128))
        wv_sb = pool.tile([128, DT, HD], F32R, tag="wv", bufs=1)
        nc.sync.dma_start(wv_sb, wvT.rearrange("(dt p) e -> p dt e", p=128))
        wo_sb = pool.tile([128, QH, D], F32R, tag="wo", bufs=1)
        C_sb = pool.tile([128, seq], F32, tag="C", bufs=1)
        nc.sync.dma_start(C_sb, Cd)
        S2_sb = pool.tile([128, seq], F32, tag="S2", bufs=1)
        nc.sync.dma_start(S2_sb, S2d)
        mask_sb = pool.tile([128, 4, SC], F32, tag="mask", bufs=1)
        nc.sync.dma_start(mask_sb, masks)
        id_sb = pool.tile([128, 128], F32, tag="id", bufs=1)
        nc.sync.dma_start(id_sb, ident)
        ones_sb = pool.tile([128, 128], F32R, tag="ones", bufs=1)
        nc.sync.dma_start(ones_sb, ones)

        pending = []
        for b in range(b_count):
            # per-chunk K/V cache tiles (avoids whole-tile WAR false deps)
            krots = [pool.tile([128, SC], F32R, tag="krot", bufs=8,
                               name=f"krot_{b}_{cc}") for cc in range(nch)]
            vs = [pool.tile([128, SC // 128, HD], F32R, tag="v", bufs=8,
                            name=f"v_{b}_{cc}") for cc in range(nch)]

            for c in range(nch):
                s0 = c * SC
                C_c = C_sb[:, s0:s0 + SC]
                S2_c = S2_sb[:, s0:s0 + SC]

                xts = []
                for dt in range(DT):
                    t = pool.tile([128, SC], F32R, tag="xt", bufs=16)
                    nc.sync.dma_start(t, xT[b, 128 * dt:128 * (dt + 1),
                                            s0:s0 + SC])
                    xts.append(t)
                if b == 0 and c == 0:
                    # wo isn't needed until the first O-proj; load it after
                    # the critical-path chunk-0 tiles so projections start
                    # sooner
                    nc.sync.dma_start(wo_sb,
                                      woT.rearrange("(h p) e -> p h e", p=128))

                # ---- Q projection + RoPE (2 heads) ----
                qts = []
                for h in range(QH):
                    ps = psum.tile([128, SC], F32, tag="proj", bufs=2)
                    for dt in range(DT):
                        nc.tensor.matmul(ps, wq_sb[:, dt, HD * h:HD * (h + 1)],
                                         xts[dt],
                                         start=(dt == 0), stop=(dt == DT - 1))
                    qt = pool.tile([128, SC], F32R, tag="qrot", bufs=6)
                    _rope(nc, pool, qt, ps, C_c, S2_c, SC)
                    qts.append(qt)

                # ---- K projection + RoPE into resident K cache ----
                psk = psum.tile([128, SC], F32, tag="proj", bufs=2)
                for dt in range(DT):
                    nc.tensor.matmul(psk, wk_sb[:, dt, :], xts[dt],
                                     start=(dt == 0), stop=(dt == DT - 1))
                _rope(nc, pool, krots[c], psk, C_c, S2_c, SC)

                # ---- V^T projection, then PE-transpose into natural V ----
                psv = psum.tile([128, SC], F32, tag="proj", bufs=2)
                for dt in range(DT):
                    nc.tensor.matmul(psv, wv_sb[:, dt, :], xts[dt],
                                     start=(dt == 0), stop=(dt == DT - 1))
                vtmp = pool.tile([128, SC], F32, tag="vtmp", bufs=2)
                nc.vector.tensor_copy(vtmp, psv)
                for st in range(SC // 128):
                    pst = psum.tile([128, 128], F32, tag="proj", bufs=2)
                    nc.tensor.transpose(pst, vtmp[:, 128 * st:128 * (st + 1)],
                                        id_sb)
                    nc.vector.tensor_copy(vs[c][:, st, :], pst)

                # ---- causal attention for this q-chunk (heads sequential:
                # head h's reciprocal tail hides under head h+1's matmuls,
                # and the last head's under the next chunk's projections
                # thanks to the deferred O-projection below) ----
                # The ones/AV matmuls for k-tile kt are emitted 2 k-tiles
                # late so the PE (strict in-order queue) never waits on the
                # DVE-mask + ACT-exp latency of the tile it consumes.
                nkt = 4 * (c + 1)
                yts = []
                for h in range(QH):
                    sum_ps = psum.tile([128, SC], F32, tag="sum", bufs=2)
                    y_ps = psum.tile([128, SC], F32, tag="y", bufs=2)
                    ess = {}

                    def emit_sums(kt, nkt=nkt, sum_ps=sum_ps, y_ps=y_ps,
                                  ess=ess, vcache=vs):
                        nc.tensor.matmul(sum_ps, ones_sb, ess[kt],
                                         start=(kt == 0), stop=(kt == nkt - 1))
                        nc.tensor.matmul(y_ps,
                                         vcache[kt // 4][:, kt % 4, :],
                                         ess.pop(kt),
                                         start=(kt == 0), stop=(kt == nkt - 1))

                    for kt in range(nkt):
                        st_ps = psum.tile([128, SC], F32, tag="st", bufs=2)
                        nc.tensor.matmul(
                            st_ps,
                            krots[kt // 4][:, 128 * (kt % 4):128 * (kt % 4 + 1)],
                            qts[h], start=True, stop=True)
                        j = kt - 4 * c
                        if j >= 0:
                            # bias is 0 beyond column 128*(j+1); add prefix only
                            w = 128 * (j + 1)
                            nc.vector.tensor_add(st_ps[:, 0:w], st_ps[:, 0:w],
                                                 mask_sb[:, j, 0:w])
                        es = pool.tile([128, SC], F32R, tag="es", bufs=5)
                        nc.scalar.activation(es, st_ps, EXP, scale=SCALE)
                        ess[kt] = es
                        if kt >= 2:
                            emit_sums(kt - 2)
                    emit_sums(nkt - 2)
                    emit_sums(nkt - 1)
                    rec = pool.tile([128, SC], F32, tag="rec", bufs=2)
                    sumsb = pool.tile([128, SC], F32, tag="sumsb", bufs=2)
                    nc.vector.tensor_copy(sumsb, sum_ps)
                    nc.vector.reciprocal_approx_fast(rec, sumsb)
                    yt = pool.tile([128, SC], F32R, tag="yt", bufs=6)
                    nc.vector.tensor_mul(yt, y_ps, rec)
                    yts.append(yt)

                # ---- deferred O projection: emit the PREVIOUS chunk's
                # O-proj here so its inputs (previous yts) are long ready
                # and this chunk's reciprocal tail hides under the next
                # chunk's projection matmuls ----
                pending.append((b, s0, yts))
                if len(pending) > 1:
                    _emit_oproj(nc, pool, psum, wo_sb, outT, *pending.pop(0))
        while pending:
            _emit_oproj(nc, pool, psum, wo_sb, outT, *pending.pop(0))

    nc.compile()
    return nc


def host_tables(seq=S):
    inv = 1.0 / (10000.0 ** (np.arange(0, HD, 2, dtype=np.float64) / HD))
    t = np.arange(seq, dtype=np.float64)
    fr = np.outer(t, inv)
    cosT = np.cos(fr).T.astype(np.float32)   # [64, seq]
    sinT = np.sin(fr).T.astype(np.float32)
    C = np.concatenate([cosT, cosT], axis=0)        # [128, seq]
    S2 = np.concatenate([sinT, -sinT], axis=0)      # [128, seq]
    masks = np.zeros((128, 4, SC), np.float32)      # additive bias, 0 = keep
    for j in range(4):
        for kk in range(128):
            masks[kk, j, :min(128 * j + kk, SC)] = NEG
    return C, S2, masks


def make_in_maps(x, wq, wk, wv, wo):
    x = np.asarray(x, np.float32)
    wq = np.asarray(wq, np.float32)
    wk = np.asarray(wk, np.float32)
    wv = np.asarray(wv, np.float32)
    wo = np.asarray(wo, np.float32)
    seq = x.shape[1]
    xT = np.ascontiguousarray(x.transpose(0, 2, 1))
    C, S2, masks = host_tables(seq)
    ident = np.eye(128, dtype=np.float32)
    ones = np.ones((128, 128), np.float32)
    in_maps = []
    for t in range(N_CORES):
        g = t // 2
        in_maps.append({
            "xT": xT,
            "wqT": np.ascontiguousarray(wq[EQ * t:EQ * (t + 1)].T),
            "wkT": np.ascontiguousarray(wk[HD * g:HD * (g + 1)].T),
            "wvT": np.ascontiguousarray(wv[HD * g:HD * (g + 1)].T),
            "woT": np.ascontiguousarray(wo[:, EQ * t:EQ * (t + 1)].T),
            "C": C, "S2": S2, "masks": masks,
            "ident": ident, "ones": ones,
        })
    return in_maps


_CACHE = {}


def kernel(x, wq, wk, wv, wo, _trace=False):
    x = np.asarray(x, np.float32)
    b_count, seq = x.shape[0], x.shape[1]
    key = (b_count, seq)
    if key not in _CACHE:
        _CACHE[key] = build(b_count, seq)
    nc = _CACHE[key]
    in_maps = make_in_maps(x, wq, wk, wv, wo)
    res = run_bass_kernel_spmd(nc, in_maps, list(range(N_CORES)),
                               trace=_trace)
    acc = res.results[0]["outT"].copy()
    for t in range(1, N_CORES):
        acc += res.results[t]["outT"]
    out = np.ascontiguousarray(acc.transpose(0, 2, 1))
    if _trace:
        kernel.last_exec_time_ns = res.exec_time_ns
        kernel.last_res = res
    return out
